# revision 44
# baseline (speedup 1.0000x reference)
"""Trainium2 Bass kernel for nn_EntityExtractor (conv3x3-pool-conv3x3-pool-conv1x1-argmaxish).

Pipeline per the reference:
  obs (3,1024,1024) -> conv3x3(3->32)+b1,relu -> maxpool2 -> conv3x3(32->64)+b2,relu
  -> maxpool2 -> conv1x1(64->16)+b3, sigmoid -> blackboard[r,c] = max_k (z!=0 ? k : 0)

Sharding: H-dim across 8 cores; each core produces 32 rows of the (256,256) output.
Each core's conv1 input is pre-packed on the host (im2col-lite) into
big1d[(xo,ro,ci), w, x]: 33 windows of 6 obs rows x 3 x-phases, so conv1 is a single
[K=54, M=128(=32co x 4rows)] matmul per 512-wide half-window.  conv2 runs as
[K=128(=4row-taps x 32ci), M=128(=2rows x 64co)] matmuls with the 3 x-taps
accumulated in PSUM.  conv3 uses the activations as the stationary operand so the
output lands pixel-major ([128px, 16ch]) for the final channel-max reduction.
"""

import sys

sys.path.insert(0, "/opt/trn_rl_repo")

import numpy as np
import ml_dtypes

import concourse.bass as bass
import concourse.bacc as bacc
import concourse.mybir as mybir
import concourse.tile as tile
from concourse import bass_utils

dt = mybir.dt
F32 = dt.float32
BF16 = dt.bfloat16

N_CORES = 8
H = W = 1024
NW1 = 33          # conv1 windows per core (4 conv rows each, stride 4, 132 rows)
NW2 = 32          # conv2 windows per core (2 conv rows each) + 1 dup-source slot
GRP = 8           # window group size for chunked tiles
AF = mybir.ActivationFunctionType
OP = mybir.AluOpType


def _build_nc(debug_dumps=False, stage=5):
    nc = bacc.Bacc("TRN2", target_bir_lowering=False, debug=False,
                   num_devices=N_CORES)

    big1d = nc.dram_tensor("big1d", (54, NW1, 1024), BF16, kind="ExternalInput")
    l1d = nc.dram_tensor("l1", (54, 128), BF16, kind="ExternalInput")
    l2d = nc.dram_tensor("l2", (128, 3, 128), BF16, kind="ExternalInput")
    w3d = nc.dram_tensor("w3e", (65, 16), BF16, kind="ExternalInput")
    b1d = nc.dram_tensor("b1t", (128, 1), F32, kind="ExternalInput")
    b2d = nc.dram_tensor("b2t", (128, 1), F32, kind="ExternalInput")
    ktd = nc.dram_tensor("kt", (128, 16), F32, kind="ExternalInput")
    emd = nc.dram_tensor("em", (128, 1), F32, kind="ExternalInput")
    outd = nc.dram_tensor("out", (128, 64), F32, kind="ExternalOutput")
    if debug_dumps:
        dp1 = nc.dram_tensor("d_p1", (128, NW2, 514), BF16, kind="ExternalOutput")
        dp2 = nc.dram_tensor("d_p2", (65, NW2, 256), BF16, kind="ExternalOutput")
        dz = nc.dram_tensor("d_z", (128, 1024), F32, kind="ExternalOutput")

    with tile.TileContext(nc) as tc:
        with (
            tc.tile_pool(name="const", bufs=1) as cpool,
            tc.tile_pool(name="big", bufs=1) as bpool,
            tc.tile_pool(name="rl", bufs=5) as rlp,
            tc.tile_pool(name="px", bufs=3) as pxp,
            tc.tile_pool(name="rl2", bufs=3) as rl2p,
            tc.tile_pool(name="px2", bufs=2) as px2p,
            tc.tile_pool(name="ps1", bufs=3, space="PSUM") as ps1p,
            tc.tile_pool(name="ps2", bufs=3, space="PSUM") as ps2p,
            tc.tile_pool(name="ps3", bufs=2, space="PSUM") as ps3p,
            tc.tile_pool(name="dbg", bufs=1) as dcp,
        ):
            # ---- constants ----
            l1t = cpool.tile([54, 128], BF16, tag="l1t")
            l2t = cpool.tile([128, 3, 128], BF16, tag="l2t")
            w3t = cpool.tile([65, 16], BF16, tag="w3t")
            b1t = cpool.tile([128, 1], F32, tag="b1t")
            b2t = cpool.tile([128, 1], F32, tag="b2t")
            ktt = cpool.tile([128, 16], F32, tag="ktt")
            emt = cpool.tile([128, 1], F32, tag="emt")
            pass  # big1 chunk 0 is issued first (PE's critical path)
            nc.sync.dma_start(l2t[:], l2d[:])
            nc.sync.dma_start(w3t[:], w3d[:])
            nc.sync.dma_start(b1t[:], b1d[:])
            nc.sync.dma_start(b2t[:], b2d[:])
            nc.sync.dma_start(ktt[:], ktd[:])
            nc.sync.dma_start(emt[:], emd[:])

            # ---- persistent big tiles, chunked by window group ----
            n_grp = 5  # groups of conv1 windows: 8,8,8,8,1
            g_sizes = [GRP, GRP, GRP, GRP, 1]
            big1 = [bpool.tile([54, gs, 1024], BF16, tag=f"big1_{g}", name=f"big1_{g}")
                    for g, gs in enumerate(g_sizes)]
            # conv2 input slots: 33 (32 windows + dup source), chunked 8,8,8,8,1
            big2 = [bpool.tile([128, gs, 514], BF16, tag=f"big2_{g}", name=f"big2_{g}")
                    for g, gs in enumerate(g_sizes)]
            # conv3 input: 32 pool2 rows, chunked by 8
            p2 = [bpool.tile([65, GRP, 256], BF16, tag=f"p2_{g}", name=f"p2_{g}")
                  for g in range(4)]
            zbuf = bpool.tile([128, 1024], F32, tag="zbuf")
            mk = bpool.tile([128, 1024], F32, tag="mk")
            bb = bpool.tile([128, 64], F32, tag="bb")

            nc.sync.dma_start(big1[0][:], big1d[:, 0:GRP, :])
            nc.sync.dma_start(l1t[:], l1d[:])
            for g, gs in enumerate(g_sizes):
                if g > 0:
                    nc.sync.dma_start(big1[g][:],
                                      big1d[:, g * GRP:g * GRP + gs, :])
                # x-padding columns for conv2 input
                nc.vector.memset(big2[g][:, :, 0:1], 0.0)
                nc.vector.memset(big2[g][:, :, 513:514], 0.0)


            def conv1_group(g):
                # evac: rl holds relu(conv+b1), parity-swizzled: evens in
                # cols [0:512), odds in [512:1024) so pool-x reads packed bf16
                gs = g_sizes[g]
                pxg = pxp.tile([128, GRP, 512], BF16, tag="pxg",
                               name=f"pxg_{g}")
                for s in range(gs):
                    w = g * GRP + s
                    rl = rlp.tile([128, 1024], BF16, tag="rl",
                                  name=f"rl_{w}")
                    rlv = rl.rearrange("p (e x) -> p x e", e=2)
                    for h in range(2):
                        on_act = ((2 * w + h) % 4) != 3  # 3/4 ScalarE
                        ps = ps1p.tile([128, 512], F32, tag="c1",
                                       name=f"c1_{w}_{h}")
                        nc.tensor.matmul(ps[:], l1t[:],
                                         big1[g][:, s, h * 512:(h + 1) * 512],
                                         start=True, stop=True)
                        pv = ps.rearrange("p (x two) -> p x two", two=2)
                        dst = rlv[:, h * 256:(h + 1) * 256, :]
                        if on_act:
                            nc.scalar.activation(dst, pv[:], AF.Relu,
                                                 bias=b1t[:], scale=1.0)
                        else:
                            nc.vector.tensor_scalar(dst, pv[:], b1t[:], 0.0,
                                                    OP.add, OP.max)
                    nc.vector.tensor_tensor(pxg[:, s, :], rl[:, 0:512],
                                            rl[:, 512:1024], OP.max)
                # one batched partition-shift DMA per group, then pool-y
                pxB = pxp.tile([64, GRP, 512], BF16, tag="pxB",
                               name=f"pxB_{g}")
                half = max(1, gs // 2)
                for lo in range(0, gs, half):
                    hi = min(lo + half, gs)
                    nc.sync.dma_start(pxB[:, lo:hi, :], pxg[64:128, lo:hi, :])
                    for s in range(lo, hi):
                        nc.vector.tensor_tensor(big2[g][0:64, s, 1:513],
                                                pxg[0:64, s, :], pxB[:, s, :],
                                                OP.max)

            def dup_group(g):
                # big2[64:128, win t] <- big2[0:64, win t+1] for t in group g
                lo = g * GRP
                n_in_g = min(GRP, NW2 - lo)
                if n_in_g > 1:
                    nc.sync.dma_start(big2[g][64:128, 0:n_in_g - 1, :],
                                      big2[g][0:64, 1:n_in_g, :])
                # last window of the group reads from the next chunk
                gn = g + 1
                nc.sync.dma_start(big2[g][64:128, n_in_g - 1, :],
                                  big2[gn][0:64, 0, :])

            def conv2_group(g):
                # pairs share weight loads (dx-major order within a pair)
                px2g = px2p.tile([128, GRP, 256], BF16, tag="px2g",
                                 name=f"px2g_{g}")
                for t0 in range(g * GRP, (g + 1) * GRP, 2):
                    pss = [ps2p.tile([128, 512], F32, tag="c2",
                                     name=f"c2_{t0}_{i}") for i in range(2)]
                    for dx in range(3):
                        for i, t in enumerate((t0, t0 + 1)):
                            s = t % GRP
                            nc.tensor.matmul(pss[i], l2t[:, dx, :],
                                             big2[g][:, s, dx:dx + 512],
                                             start=(dx == 0), stop=(dx == 2))
                    for i, t in enumerate((t0, t0 + 1)):
                        s = t % GRP
                        rl2 = rl2p.tile([128, 512], BF16, tag="rl2",
                                        name=f"rl2_{t}")
                        rl2v = rl2.rearrange("p (e x) -> p x e", e=2)
                        pv = pss[i].rearrange("p (x two) -> p x two", two=2)
                        nc.scalar.activation(rl2v[:], pv[:], AF.Relu,
                                             bias=b2t[:], scale=1.0)
                        nc.vector.tensor_tensor(px2g[:, s, :], rl2[:, 0:256],
                                                rl2[:, 256:512], OP.max)
                px2B = px2p.tile([64, GRP, 256], BF16, tag="px2B",
                                 name=f"px2B_{g}")
                for lo in range(0, GRP, 4):
                    nc.sync.dma_start(px2B[:, lo:lo + 4, :],
                                      px2g[64:128, lo:lo + 4, :])
                    for s in range(lo, lo + 4):
                        nc.vector.tensor_tensor(p2[g][0:64, s, :],
                                                px2g[0:64, s, :],
                                                px2B[:, s, :], OP.max)

            def conv3_half(hh):
                ps3 = ps3p.tile([128, 512], F32, tag="c3", name=f"c3_{hh}")
                for b in range(32):
                    blk = hh * 32 + b
                    t, half = blk // 2, blk % 2
                    g, s = t // GRP, t % GRP
                    nc.tensor.matmul(ps3[:, b * 16:(b + 1) * 16],
                                     p2[g][:, s, half * 128:(half + 1) * 128],
                                     w3t[:], start=True, stop=True)
                nc.scalar.activation(zbuf[:, hh * 512:(hh + 1) * 512],
                                     ps3[:], AF.Sigmoid)
                zv = zbuf[:, hh * 512:(hh + 1) * 512].rearrange(
                    "p (b k) -> p b k", k=16)
                mkv = mk[:, hh * 512:(hh + 1) * 512].rearrange(
                    "p (b k) -> p b k", k=16)
                ktb = ktt[:, None, :].to_broadcast((128, 32, 16))
                nc.vector.scalar_tensor_tensor(mkv[:], zv[:], 0.0, ktb,
                                               OP.is_gt, OP.mult)
                nc.vector.tensor_reduce(bb[:, hh * 32:(hh + 1) * 32], mkv[:],
                                        axis=mybir.AxisListType.X, op=OP.max)

            # ---- pipeline: conv1 groups -> dups/edges -> conv2 -> conv3 ----
            if stage >= 2:
                for g in range(5):
                    conv1_group(g)
            if stage >= 3:
                nc.vector.tensor_scalar(big2[0][0:32, 0, :],
                                        big2[0][0:32, 0, :],
                                        emt[0:32, :], None, OP.mult)
                for g in range(4):
                    dup_group(g)
                nc.vector.tensor_scalar(big2[3][96:128, 7, :],
                                        big2[3][96:128, 7, :],
                                        emt[96:128, :], None, OP.mult)
            if stage >= 4:
                for g in range(4):
                    nc.vector.memset(p2[g][64:65, :, :], 1.0)  # bias row
                    conv2_group(g)
            if stage >= 5:
                for hh in range(2):
                    conv3_half(hh)
            else:
                nc.vector.memset(bb[:], 0.0)
            nc.sync.dma_start(outd[:], bb[:])

            if debug_dumps:
                for g in range(4):
                    nc.sync.dma_start(dp1[:, g * GRP:(g + 1) * GRP, :], big2[g][:])
                    nc.sync.dma_start(dp2[:, g * GRP:(g + 1) * GRP, :], p2[g][:])
                nc.sync.dma_start(dz[:], zbuf[:])

    nc.compile()
    return nc


def _prep_weights(w1, b1, w2, b2, w3, b3):
    """Host-side weight layout transforms (numpy)."""
    bf = ml_dtypes.bfloat16
    # conv1 lhsT: [54=(xo,ro,ci), 128=M], M = ((j&1)*2 + (j>>1))*32 + co
    l1 = np.zeros((54, 128), np.float32)
    for xo in range(3):
        for ro in range(6):
            for j in range(4):
                dy = ro - j
                if 0 <= dy <= 2:
                    m0 = ((j & 1) * 2 + (j >> 1)) * 32
                    for ci in range(3):
                        l1[xo * 18 + ro * 3 + ci, m0:m0 + 32] = w1[:, ci, dy, xo]
    # conv2 lhsT: [128=(ro,ci), 3=dx, 128=(j,co)]
    l2 = np.zeros((128, 3, 128), np.float32)
    for ro in range(4):
        for j in range(2):
            dy = ro - j
            if 0 <= dy <= 2:
                for ci in range(32):
                    # w2[co, ci, dy, dx] -> l2[ro*32+ci, dx, j*64+co]
                    l2[ro * 32 + ci, :, j * 64:j * 64 + 64] = w2[:, ci, dy, :].T
    # conv3 rhs with bias row
    w3e = np.zeros((65, 16), np.float32)
    w3e[:64] = w3[:, :, 0, 0].T
    w3e[64] = b3
    b1t = np.tile(b1, 4).astype(np.float32).reshape(128, 1)
    b2t = np.tile(b2, 2).astype(np.float32).reshape(128, 1)
    kt = np.tile(np.arange(16, dtype=np.float32), (128, 1))
    return (l1.astype(bf), l2.astype(bf), w3e.astype(bf),
            np.ascontiguousarray(b1t), np.ascontiguousarray(b2t), kt)


def _prep_big1(obs):
    """Pack obs into per-core conv1 input tensors big1d[(xo,ro,ci), w, x]."""
    bf = ml_dtypes.bfloat16
    obs_p = np.zeros((3, H + 6, W + 2), dtype=bf)
    obs_p[:, 3:3 + H, 1:1 + W] = obs.astype(bf)
    out = []
    r_idx = 4 * np.arange(NW1)[:, None] + np.arange(6)[None, :]  # (33, 6)
    for c in range(N_CORES):
        sub = obs_p[:, 128 * c + r_idx, :]          # (3ci, 33w, 6ro, 1026)
        stack = np.stack([sub[..., xo:xo + W] for xo in range(3)], axis=0)
        # (xo, ci, w, ro, x) -> (xo, ro, ci, w, x)
        big1d = stack.transpose(0, 3, 1, 2, 4).reshape(54, NW1, W)
        out.append(np.ascontiguousarray(big1d))
    return out


_NC_CACHE = {}


def kernel(obs, w1, b1, w2, b2, w3, b3):
    obs = np.asarray(obs, dtype=np.float32)
    w1, b1 = np.asarray(w1, np.float32), np.asarray(b1, np.float32)
    w2, b2 = np.asarray(w2, np.float32), np.asarray(b2, np.float32)
    w3, b3 = np.asarray(w3, np.float32), np.asarray(b3, np.float32)

    if "nc" not in _NC_CACHE:
        _NC_CACHE["nc"] = _build_nc()
    nc = _NC_CACHE["nc"]

    l1, l2, w3e, b1t, b2t, kt = _prep_weights(w1, b1, w2, b2, w3, b3)
    big1s = _prep_big1(obs)

    in_maps = []
    for c in range(N_CORES):
        em = np.ones((128, 1), np.float32)
        if c == 0:
            em[0:32] = 0.0
        if c == N_CORES - 1:
            em[96:128] = 0.0
        in_maps.append({
            "big1d": big1s[c], "l1": l1, "l2": l2, "w3e": w3e,
            "b1t": b1t, "b2t": b2t, "kt": kt, "em": em,
        })

    last_err = None
    for attempt in range(3):
        try:
            res = bass_utils.run_bass_kernel_spmd(
                nc, in_maps, core_ids=list(range(N_CORES)))
            outs = []
            for c in range(N_CORES):
                a = np.asarray(res.results[c]["out"])  # [128 px, 64 blk]
                outs.append(a.T.reshape(32, 256))
            return np.concatenate(outs, axis=0).astype(np.float32)
        except Exception as e:  # transient device wedges; retry fresh
            last_err = e
            _NC_CACHE.clear()
            _NC_CACHE["nc"] = nc = _build_nc()
    raise last_err


# revision 50
# speedup vs baseline: 1.0062x; 1.0062x over previous
"""Trainium2 Bass kernel for nn_EntityExtractor (conv3x3-pool-conv3x3-pool-conv1x1-argmaxish).

Pipeline per the reference:
  obs (3,1024,1024) -> conv3x3(3->32)+b1,relu -> maxpool2 -> conv3x3(32->64)+b2,relu
  -> maxpool2 -> conv1x1(64->16)+b3, sigmoid -> blackboard[r,c] = max_k (z!=0 ? k : 0)

Sharding: H-dim across 8 cores; each core produces 32 rows of the (256,256) output.
Each core's conv1 input is pre-packed on the host (im2col-lite) into
big1d[(xo,ro,ci), w, x]: 33 windows of 6 obs rows x 3 x-phases, so conv1 is a single
[K=54, M=128(=32co x 4rows)] matmul per 512-wide half-window.  conv2 runs as
[K=128(=4row-taps x 32ci), M=128(=2rows x 64co)] matmuls with the 3 x-taps
accumulated in PSUM.  conv3 uses the activations as the stationary operand so the
output lands pixel-major ([128px, 16ch]) for the final channel-max reduction.
"""

import sys

sys.path.insert(0, "/opt/trn_rl_repo")

import numpy as np
import ml_dtypes

import concourse.bass as bass
import concourse.bacc as bacc
import concourse.mybir as mybir
import concourse.tile as tile
from concourse import bass_utils

dt = mybir.dt
F32 = dt.float32
BF16 = dt.bfloat16

N_CORES = 8
H = W = 1024
NW1 = 33          # conv1 windows per core (4 conv rows each, stride 4, 132 rows)
NW2 = 32          # conv2 windows per core (2 conv rows each) + 1 dup-source slot
GRP = 8           # window group size for chunked tiles
AF = mybir.ActivationFunctionType
OP = mybir.AluOpType


def _build_nc(debug_dumps=False, stage=5):
    nc = bacc.Bacc("TRN2", target_bir_lowering=False, debug=False,
                   num_devices=N_CORES)

    big1d = nc.dram_tensor("big1d", (54, NW1, 1024), BF16, kind="ExternalInput")
    l1d = nc.dram_tensor("l1", (54, 128), BF16, kind="ExternalInput")
    l2d = nc.dram_tensor("l2", (128, 3, 128), BF16, kind="ExternalInput")
    w3d = nc.dram_tensor("w3e", (65, 16), BF16, kind="ExternalInput")
    b1d = nc.dram_tensor("b1t", (128, 1), F32, kind="ExternalInput")
    b2d = nc.dram_tensor("b2t", (128, 1), F32, kind="ExternalInput")
    ktd = nc.dram_tensor("kt", (128, 16), F32, kind="ExternalInput")
    emd = nc.dram_tensor("em", (128, 1), F32, kind="ExternalInput")
    outd = nc.dram_tensor("out", (128, 64), F32, kind="ExternalOutput")
    if debug_dumps:
        dp1 = nc.dram_tensor("d_p1", (128, NW2, 514), BF16, kind="ExternalOutput")
        dp2 = nc.dram_tensor("d_p2", (65, NW2, 256), BF16, kind="ExternalOutput")
        dz = nc.dram_tensor("d_z", (128, 1024), F32, kind="ExternalOutput")

    with tile.TileContext(nc) as tc:
        with (
            tc.tile_pool(name="const", bufs=1) as cpool,
            tc.tile_pool(name="big", bufs=1) as bpool,
            tc.tile_pool(name="rl", bufs=5) as rlp,
            tc.tile_pool(name="px", bufs=3) as pxp,
            tc.tile_pool(name="rl2", bufs=3) as rl2p,
            tc.tile_pool(name="px2", bufs=2) as px2p,
            tc.tile_pool(name="ps1", bufs=3, space="PSUM") as ps1p,
            tc.tile_pool(name="ps2", bufs=3, space="PSUM") as ps2p,
            tc.tile_pool(name="ps3", bufs=2, space="PSUM") as ps3p,
            tc.tile_pool(name="dbg", bufs=1) as dcp,
        ):
            # ---- constants ----
            l1t = cpool.tile([54, 128], BF16, tag="l1t")
            l2t = cpool.tile([128, 3, 128], BF16, tag="l2t")
            w3t = cpool.tile([65, 16], BF16, tag="w3t")
            b1t = cpool.tile([128, 1], F32, tag="b1t")
            b2t = cpool.tile([128, 1], F32, tag="b2t")
            ktt = cpool.tile([128, 16], F32, tag="ktt")
            emt = cpool.tile([128, 1], F32, tag="emt")
            pass  # big1 chunk 0 is issued first (PE's critical path)
            nc.sync.dma_start(l2t[:], l2d[:])
            nc.sync.dma_start(w3t[:], w3d[:])
            nc.sync.dma_start(b1t[:], b1d[:])
            nc.sync.dma_start(b2t[:], b2d[:])
            nc.sync.dma_start(ktt[:], ktd[:])
            nc.sync.dma_start(emt[:], emd[:])

            # ---- persistent big tiles, chunked by window group ----
            n_grp = 5  # groups of conv1 windows: 8,8,8,8,1
            g_sizes = [GRP, GRP, GRP, GRP, 1]
            big1 = [bpool.tile([54, gs, 1024], BF16, tag=f"big1_{g}", name=f"big1_{g}")
                    for g, gs in enumerate(g_sizes)]
            # conv2 input slots: 33 (32 windows + dup source), chunked 8,8,8,8,1
            big2 = [bpool.tile([128, gs, 514], BF16, tag=f"big2_{g}", name=f"big2_{g}")
                    for g, gs in enumerate(g_sizes)]
            # conv3 input: 32 pool2 rows, chunked by 8
            p2 = [bpool.tile([65, GRP, 256], BF16, tag=f"p2_{g}", name=f"p2_{g}")
                  for g in range(4)]
            zbuf = bpool.tile([128, 1024], F32, tag="zbuf")
            mk = bpool.tile([128, 1024], F32, tag="mk")
            bb = bpool.tile([128, 64], F32, tag="bb")

            nc.sync.dma_start(big1[0][:], big1d[:, 0:GRP, :])
            nc.sync.dma_start(l1t[:], l1d[:])
            for g, gs in enumerate(g_sizes):
                if g > 0:
                    nc.sync.dma_start(big1[g][:],
                                      big1d[:, g * GRP:g * GRP + gs, :])
                # x-padding columns for conv2 input
                nc.vector.memset(big2[g][:, :, 0:1], 0.0)
                nc.vector.memset(big2[g][:, :, 513:514], 0.0)


            def conv1_group(g):
                # evac: rl holds relu(conv+b1), parity-swizzled: evens in
                # cols [0:512), odds in [512:1024) so pool-x reads packed bf16
                gs = g_sizes[g]
                pxg = pxp.tile([128, GRP, 512], BF16, tag="pxg",
                               name=f"pxg_{g}")
                for s in range(gs):
                    w = g * GRP + s
                    rl = rlp.tile([128, 1024], BF16, tag="rl",
                                  name=f"rl_{w}")
                    rlv = rl.rearrange("p (e x) -> p x e", e=2)
                    for h in range(2):
                        on_act = ((2 * w + h) % 8) != 7  # 7/8 ScalarE
                        ps = ps1p.tile([128, 512], F32, tag="c1",
                                       name=f"c1_{w}_{h}")
                        nc.tensor.matmul(ps[:], l1t[:],
                                         big1[g][:, s, h * 512:(h + 1) * 512],
                                         start=True, stop=True)
                        pv = ps.rearrange("p (x two) -> p x two", two=2)
                        dst = rlv[:, h * 256:(h + 1) * 256, :]
                        if on_act:
                            nc.scalar.activation(dst, pv[:], AF.Relu,
                                                 bias=b1t[:], scale=1.0)
                        else:
                            nc.vector.tensor_scalar(dst, pv[:], b1t[:], 0.0,
                                                    OP.add, OP.max)
                    nc.vector.tensor_tensor(pxg[:, s, :], rl[:, 0:512],
                                            rl[:, 512:1024], OP.max)
                # one batched partition-shift DMA per group, then pool-y
                pxB = pxp.tile([64, GRP, 512], BF16, tag="pxB",
                               name=f"pxB_{g}")
                half = max(1, gs // 2)
                for lo in range(0, gs, half):
                    hi = min(lo + half, gs)
                    nc.sync.dma_start(pxB[:, lo:hi, :], pxg[64:128, lo:hi, :])
                    for s in range(lo, hi):
                        nc.vector.tensor_tensor(big2[g][0:64, s, 1:513],
                                                pxg[0:64, s, :], pxB[:, s, :],
                                                OP.max)

            def dup_group(g):
                # big2[64:128, win t] <- big2[0:64, win t+1] for t in group g
                lo = g * GRP
                n_in_g = min(GRP, NW2 - lo)
                if n_in_g > 1:
                    nc.sync.dma_start(big2[g][64:128, 0:n_in_g - 1, :],
                                      big2[g][0:64, 1:n_in_g, :])
                # last window of the group reads from the next chunk
                gn = g + 1
                nc.sync.dma_start(big2[g][64:128, n_in_g - 1, :],
                                  big2[gn][0:64, 0, :])

            def conv2_group(g):
                # pairs share weight loads (dx-major order within a pair)
                px2g = px2p.tile([128, GRP, 256], BF16, tag="px2g",
                                 name=f"px2g_{g}")
                for t0 in range(g * GRP, (g + 1) * GRP, 2):
                    pss = [ps2p.tile([128, 512], F32, tag="c2",
                                     name=f"c2_{t0}_{i}") for i in range(2)]
                    for dx in range(3):
                        for i, t in enumerate((t0, t0 + 1)):
                            s = t % GRP
                            nc.tensor.matmul(pss[i], l2t[:, dx, :],
                                             big2[g][:, s, dx:dx + 512],
                                             start=(dx == 0), stop=(dx == 2))
                    for i, t in enumerate((t0, t0 + 1)):
                        s = t % GRP
                        rl2 = rl2p.tile([128, 512], BF16, tag="rl2",
                                        name=f"rl2_{t}")
                        rl2v = rl2.rearrange("p (e x) -> p x e", e=2)
                        pv = pss[i].rearrange("p (x two) -> p x two", two=2)
                        nc.scalar.activation(rl2v[:], pv[:], AF.Relu,
                                             bias=b2t[:], scale=1.0)
                        nc.vector.tensor_tensor(px2g[:, s, :], rl2[:, 0:256],
                                                rl2[:, 256:512], OP.max)
                px2B = px2p.tile([64, GRP, 256], BF16, tag="px2B",
                                 name=f"px2B_{g}")
                for lo in range(0, GRP, 4):
                    nc.sync.dma_start(px2B[:, lo:lo + 4, :],
                                      px2g[64:128, lo:lo + 4, :])
                    for s in range(lo, lo + 4):
                        nc.vector.tensor_tensor(p2[g][0:64, s, :],
                                                px2g[0:64, s, :],
                                                px2B[:, s, :], OP.max)

            def conv3_half(hh):
                ps3 = ps3p.tile([128, 512], F32, tag="c3", name=f"c3_{hh}")
                for b in range(32):
                    blk = hh * 32 + b
                    t, half = blk // 2, blk % 2
                    g, s = t // GRP, t % GRP
                    nc.tensor.matmul(ps3[:, b * 16:(b + 1) * 16],
                                     p2[g][:, s, half * 128:(half + 1) * 128],
                                     w3t[:], start=True, stop=True)
                nc.scalar.activation(zbuf[:, hh * 512:(hh + 1) * 512],
                                     ps3[:], AF.Sigmoid)
                zv = zbuf[:, hh * 512:(hh + 1) * 512].rearrange(
                    "p (b k) -> p b k", k=16)
                mkv = mk[:, hh * 512:(hh + 1) * 512].rearrange(
                    "p (b k) -> p b k", k=16)
                ktb = ktt[:, None, :].to_broadcast((128, 32, 16))
                nc.vector.scalar_tensor_tensor(mkv[:], zv[:], 0.0, ktb,
                                               OP.is_gt, OP.mult)
                nc.vector.tensor_reduce(bb[:, hh * 32:(hh + 1) * 32], mkv[:],
                                        axis=mybir.AxisListType.X, op=OP.max)

            # ---- pipeline: conv1 groups -> dups/edges -> conv2 -> conv3 ----
            if stage >= 2:
                for g in range(5):
                    conv1_group(g)
            if stage >= 3:
                nc.vector.tensor_scalar(big2[0][0:32, 0, :],
                                        big2[0][0:32, 0, :],
                                        emt[0:32, :], None, OP.mult)
                for g in range(4):
                    dup_group(g)
                nc.vector.tensor_scalar(big2[3][96:128, 7, :],
                                        big2[3][96:128, 7, :],
                                        emt[96:128, :], None, OP.mult)
            if stage >= 4:
                for g in range(4):
                    nc.vector.memset(p2[g][64:65, :, :], 1.0)  # bias row
                    conv2_group(g)
            if stage >= 5:
                for hh in range(2):
                    conv3_half(hh)
            else:
                nc.vector.memset(bb[:], 0.0)
            nc.sync.dma_start(outd[:], bb[:])

            if debug_dumps:
                for g in range(4):
                    nc.sync.dma_start(dp1[:, g * GRP:(g + 1) * GRP, :], big2[g][:])
                    nc.sync.dma_start(dp2[:, g * GRP:(g + 1) * GRP, :], p2[g][:])
                nc.sync.dma_start(dz[:], zbuf[:])

    nc.compile()
    return nc


def _prep_weights(w1, b1, w2, b2, w3, b3):
    """Host-side weight layout transforms (numpy)."""
    bf = ml_dtypes.bfloat16
    # conv1 lhsT: [54=(xo,ro,ci), 128=M], M = ((j&1)*2 + (j>>1))*32 + co
    l1 = np.zeros((54, 128), np.float32)
    for xo in range(3):
        for ro in range(6):
            for j in range(4):
                dy = ro - j
                if 0 <= dy <= 2:
                    m0 = ((j & 1) * 2 + (j >> 1)) * 32
                    for ci in range(3):
                        l1[xo * 18 + ro * 3 + ci, m0:m0 + 32] = w1[:, ci, dy, xo]
    # conv2 lhsT: [128=(ro,ci), 3=dx, 128=(j,co)]
    l2 = np.zeros((128, 3, 128), np.float32)
    for ro in range(4):
        for j in range(2):
            dy = ro - j
            if 0 <= dy <= 2:
                for ci in range(32):
                    # w2[co, ci, dy, dx] -> l2[ro*32+ci, dx, j*64+co]
                    l2[ro * 32 + ci, :, j * 64:j * 64 + 64] = w2[:, ci, dy, :].T
    # conv3 rhs with bias row
    w3e = np.zeros((65, 16), np.float32)
    w3e[:64] = w3[:, :, 0, 0].T
    w3e[64] = b3
    b1t = np.tile(b1, 4).astype(np.float32).reshape(128, 1)
    b2t = np.tile(b2, 2).astype(np.float32).reshape(128, 1)
    kt = np.tile(np.arange(16, dtype=np.float32), (128, 1))
    return (l1.astype(bf), l2.astype(bf), w3e.astype(bf),
            np.ascontiguousarray(b1t), np.ascontiguousarray(b2t), kt)


def _prep_big1(obs):
    """Pack obs into per-core conv1 input tensors big1d[(xo,ro,ci), w, x]."""
    bf = ml_dtypes.bfloat16
    obs_p = np.zeros((3, H + 6, W + 2), dtype=bf)
    obs_p[:, 3:3 + H, 1:1 + W] = obs.astype(bf)
    out = []
    r_idx = 4 * np.arange(NW1)[:, None] + np.arange(6)[None, :]  # (33, 6)
    for c in range(N_CORES):
        sub = obs_p[:, 128 * c + r_idx, :]          # (3ci, 33w, 6ro, 1026)
        stack = np.stack([sub[..., xo:xo + W] for xo in range(3)], axis=0)
        # (xo, ci, w, ro, x) -> (xo, ro, ci, w, x)
        big1d = stack.transpose(0, 3, 1, 2, 4).reshape(54, NW1, W)
        out.append(np.ascontiguousarray(big1d))
    return out


_NC_CACHE = {}


def kernel(obs, w1, b1, w2, b2, w3, b3):
    obs = np.asarray(obs, dtype=np.float32)
    w1, b1 = np.asarray(w1, np.float32), np.asarray(b1, np.float32)
    w2, b2 = np.asarray(w2, np.float32), np.asarray(b2, np.float32)
    w3, b3 = np.asarray(w3, np.float32), np.asarray(b3, np.float32)

    if "nc" not in _NC_CACHE:
        _NC_CACHE["nc"] = _build_nc()
    nc = _NC_CACHE["nc"]

    l1, l2, w3e, b1t, b2t, kt = _prep_weights(w1, b1, w2, b2, w3, b3)
    big1s = _prep_big1(obs)

    in_maps = []
    for c in range(N_CORES):
        em = np.ones((128, 1), np.float32)
        if c == 0:
            em[0:32] = 0.0
        if c == N_CORES - 1:
            em[96:128] = 0.0
        in_maps.append({
            "big1d": big1s[c], "l1": l1, "l2": l2, "w3e": w3e,
            "b1t": b1t, "b2t": b2t, "kt": kt, "em": em,
        })

    last_err = None
    for attempt in range(3):
        try:
            res = bass_utils.run_bass_kernel_spmd(
                nc, in_maps, core_ids=list(range(N_CORES)))
            outs = []
            for c in range(N_CORES):
                a = np.asarray(res.results[c]["out"])  # [128 px, 64 blk]
                outs.append(a.T.reshape(32, 256))
            return np.concatenate(outs, axis=0).astype(np.float32)
        except Exception as e:  # transient device wedges; retry fresh
            last_err = e
            _NC_CACHE.clear()
            _NC_CACHE["nc"] = nc = _build_nc()
    raise last_err
